# revision 1
# baseline (speedup 1.0000x reference)
"""Trainium2 Bass kernel for nn_MultiHeadAttention (B=2, S=2048, D=1024, H=16).

Sharding (8 cores): data-parallel over batch (2) x tensor-parallel over
head groups (4 groups of 4 heads). Core c handles batch c//4, heads
4*(c%4) .. 4*(c%4)+3.  Each core computes the full attention for its
heads plus its slice of the output projection; the host sums the 4
partial output projections per batch and adds bo.

On-chip layouts (per core):
  qT, kT  [256 feat, 2048 seq]   (features on partitions)
  v       [2048 keys, 4*65]      (per head: 64 feats + ones column)
  scoresT [keys, queries] tiles -> exp on the scalar engine with the
          1/sqrt(64) scale fused (max-subtract skipped: softmax is
          shift invariant and scores are O(1) here)
  ctxT    [65, queries] accumulated over key tiles; row 64 = sum of exp
          (from the ones column) -> broadcast -> reciprocal -> scale.
All matmuls run in bf16 with fp32 PSUM accumulation; inputs are cast to
bf16 on the host (halves HBM traffic, enables fast weight loads).

Schedule: the q/k/v projections are emitted as <=4-matmul "filler
granules" popped between attention chunks so they hide in the scalar
engine (exp) bound attention phase.  Tile dependencies are trace-order
based, so every producer granule pops before its first consumer is
emitted.  ctx matmuls for j==0 are deferred (exp tiles buffered) until
the interleaved v projection has produced the needed v tiles.
"""

import sys

for _p in ("/opt/trn_rl_repo",):
    if _p not in sys.path:
        sys.path.insert(0, _p)

from contextlib import ExitStack

import ml_dtypes
import numpy as np

import concourse.bass as bass
import concourse.tile as tile
from concourse import bacc, mybir
from concourse.bass_utils import run_bass_kernel_spmd

B, S, D, H = 2, 2048, 1024, 16
HD = D // H            # 64 head dim
NG = 4                 # head groups (cores per batch)
NHC = H // NG          # 4 heads per core
FS = NHC * HD          # 256 features per core
P = 128
DK = D // P            # 8 contraction tiles for projections
SK = S // P            # 16 key tiles
NQ = S // 512          # 4 query chunks
FK = FS // P           # 2 feature tiles for qT/kT/ctxT
VW = HD + 1            # v feats + ones column

f32 = mybir.dt.float32
bf16 = mybir.dt.bfloat16
EXP = mybir.ActivationFunctionType.Exp
CHUNKS = (2,) * 8   # key tiles per exp chunk (16 total)


def _emit(ctx: ExitStack, tc, nc, io):
    QT, KT, VT, WqT, WkT, WvT, WoT, bq, bk, bv, OUTP = io

    xt = ctx.enter_context(tc.tile_pool(name="xt", bufs=24))
    wp = ctx.enter_context(tc.tile_pool(name="wp", bufs=1))
    per = ctx.enter_context(tc.tile_pool(name="per", bufs=1))
    exp = ctx.enter_context(tc.tile_pool(name="exp", bufs=18))
    nrm = ctx.enter_context(tc.tile_pool(name="nrm", bufs=2))
    ctxp = ctx.enter_context(tc.tile_pool(name="ctxp", bufs=2))
    outp = ctx.enter_context(tc.tile_pool(name="outp", bufs=4))
    sc_ps = ctx.enter_context(tc.tile_pool(name="sc_ps", bufs=2, space="PSUM"))
    acc_ps = ctx.enter_context(tc.tile_pool(name="acc_ps", bufs=4, space="PSUM"))

    # ---- weights / biases (persistent) ----
    wq = [wp.tile([P, FS], bf16, tag=f"wq{k}", name=f"wq{k}") for k in range(DK)]
    wk = [wp.tile([P, FS], bf16, tag=f"wk{k}", name=f"wk{k}") for k in range(DK)]
    wv = [wp.tile([P, FS], bf16, tag=f"wv{k}", name=f"wv{k}") for k in range(DK)]
    wo = [wp.tile([P, D], bf16, tag=f"wo{f}", name=f"wo{f}") for f in range(FK)]
    for k in range(DK):
        nc.sync.dma_start(wk[k][:], WkT[k * P:(k + 1) * P, :])
    bq_t = [wp.tile([P, 1], f32, tag=f"bq{f}", name=f"bqt{f}") for f in range(FK)]
    bk_t = [wp.tile([P, 1], f32, tag=f"bk{f}", name=f"bkt{f}") for f in range(FK)]
    for f in range(FK):
        nc.sync.dma_start(bq_t[f][:], bq[f * P:(f + 1) * P, :])
        nc.sync.dma_start(bk_t[f][:], bk[f * P:(f + 1) * P, :])
    bv_t = wp.tile([P, FS], f32, tag="bv")
    nc.sync.dma_start(bv_t[:], bv.to_broadcast((P, FS)))
    ones_t = wp.tile([P, NHC], f32, tag="ones")
    nc.vector.memset(ones_t[:], 1.0)

    # ---- persistent activations ----
    kT = [per.tile([P, S], bf16, tag=f"kT{f}", name=f"kTs{f}") for f in range(FK)]
    qT = [per.tile([P, S], bf16, tag=f"qT{f}", name=f"qTs{f}") for f in range(FK)]
    vsb = [per.tile([P, NHC * VW], bf16, tag=f"v{t}", name=f"vs{t}")
           for t in range(SK)]

    # ---- input streaming: [128, 1024] bf16 tiles ----
    def load_half(src, hf, eng=None):
        tiles = {}
        for k in range(DK):
            t = xt.tile([P, 1024], bf16, tag="xt", name="xtile")
            (eng or nc.sync).dma_start(t[:], src[k * P:(k + 1) * P,
                                                 hf * 1024:(hf + 1) * 1024])
            tiles[k] = t
        return tiles

    def proj_cols(src_tiles, w, b_t, dst, ncol):
        # dst[f][:, ncol*512:+512] = (W_slice @ X^T + b)
        off = (ncol * 512) % 1024
        for f in range(FK):
            ps = acc_ps.tile([P, 512], f32, tag="acc")
            for k in range(DK):
                nc.tensor.matmul(
                    ps[:],
                    w[k][:, f * P:(f + 1) * P],
                    src_tiles[k][:, off:off + 512],
                    start=(k == 0), stop=(k == DK - 1),
                )
            nc.vector.tensor_scalar_add(
                dst[f][:, ncol * 512:(ncol + 1) * 512], ps[:], b_t[f][:])

    # ---- emission order tuned for overlap ----
    kt_h = [load_half(KT, 0), load_half(KT, 1)]
    for k in range(DK):
        nc.gpsimd.dma_start(wq[k][:], WqT[k * P:(k + 1) * P, :])
    qt_h0 = load_half(QT, 0, nc.gpsimd)
    vt_h = [load_half(VT, 0, nc.scalar), load_half(VT, 1, nc.scalar)]
    for k in range(DK):
        nc.sync.dma_start(wv[k][:], WvT[k * P:(k + 1) * P, :])
    for f in range(FK):
        nc.sync.dma_start(wo[f][:], WoT[f * P:(f + 1) * P, :])
    qt_h1 = load_half(QT, 1, nc.gpsimd)

    # HAM pre-warm: ~5us of dependency-free matmuls while the first DMAs
    # land, so real matmuls start at 2.4 GHz instead of 1.2 GHz.
    warm_sb = wp.tile([P, 16], f32, tag="warm")
    nc.vector.memset(warm_sb[:], 0.0)
    warm_ps = acc_ps.tile([16, 16], f32, tag="acc", name="warmps")
    for _ in range(100):
        nc.tensor.matmul(warm_ps[:], warm_sb[:, 0:16], warm_sb[:],
                         start=True, stop=True)
    warm_ex = wp.tile([P, 16], bf16, tag="warmex")
    nc.scalar.activation(warm_ex[:], warm_sb[:], EXP, scale=0.125)

    proj_cols(kt_h[0], wk, bk_t, kT, 0)
    proj_cols(qt_h0, wq, bq_t, qT, 0)

    # ---- filler granules (<=4 matmuls each) ----
    def proj_granules(src_tiles, w, b_t, dst, ncol, f):
        off = (ncol * 512) % 1024
        st = {}

        def g0():
            st["ps"] = acc_ps.tile([P, 512], f32, tag="acc", name="pps")
            for k in range(4):
                nc.tensor.matmul(
                    st["ps"][:], w[k][:, f * P:(f + 1) * P],
                    src_tiles[k][:, off:off + 512],
                    start=(k == 0), stop=False)

        def g1():
            for k in range(4, DK):
                nc.tensor.matmul(
                    st["ps"][:], w[k][:, f * P:(f + 1) * P],
                    src_tiles[k][:, off:off + 512],
                    start=False, stop=(k == DK - 1))
            nc.vector.tensor_scalar_add(
                dst[f][:, ncol * 512:(ncol + 1) * 512], st["ps"][:], b_t[f][:])
        return [g0, g1]

    def vproj_granules(t):
        st = {}

        def g0():
            st["ps"] = acc_ps.tile([P, FS], f32, tag="acc", name="vps")
            for k in range(4):
                nc.tensor.matmul(
                    st["ps"][:],
                    vt_h[t * P // 1024][k][:, (t * P) % 1024:(t * P) % 1024 + P],
                    wv[k][:], start=(k == 0), stop=False)

        def g1():
            for k in range(4, DK):
                nc.tensor.matmul(
                    st["ps"][:],
                    vt_h[t * P // 1024][k][:, (t * P) % 1024:(t * P) % 1024 + P],
                    wv[k][:], start=False, stop=(k == DK - 1))
            for h in range(NHC):
                nc.vector.tensor_add(
                    vsb[t][:, h * VW:h * VW + HD],
                    st["ps"][:, h * HD:(h + 1) * HD],
                    bv_t[:, h * HD:(h + 1) * HD])
            nc.vector.tensor_copy(vsb[t][:, HD:NHC * VW:VW], ones_t[:])
        return [g0, g1]

    def outproj_unit(j, ctxn, mt, oc):
        def emit():
            ps = acc_ps.tile([P, 512], f32, tag="acc", name="ops")
            for f in range(FK):
                nc.tensor.matmul(
                    ps[:], ctxn[f][:, mt * P:(mt + 1) * P],
                    wo[f][:, oc * 512:(oc + 1) * 512],
                    start=(f == 0), stop=(f == FK - 1))
            ob = outp.tile([P, 512], f32, tag="ob")
            nc.vector.tensor_copy(ob[:], ps[:])
            nc.sync.dma_start(
                OUTP[j * 512 + mt * P: j * 512 + (mt + 1) * P,
                     oc * 512:(oc + 1) * 512], ob[:])
        return emit

    def outproj_units(j, ctxn):
        return [outproj_unit(j, ctxn, mt, oc)
                for mt in range(4) for oc in range(2)]

    # NOTE: Tile dependencies are trace-order based -- every producer must
    # be EMITTED before its consumer.  All projection fillers therefore pop
    # inside j==0 (before any j>=1 instruction is traced).
    j0_fill = []
    for ncol in (1, 2, 3):
        for f in range(FK):
            j0_fill += proj_granules(kt_h[ncol // 2], wk, bk_t, kT, ncol, f)
    n_vp_before = len(j0_fill)  # 12 kT granules popped first
    for t in range(SK):
        j0_fill += vproj_granules(t)
    for f in range(FK):
        j0_fill += proj_granules(qt_h0, wq, bq_t, qT, 1, f)
    jn_fill = {1: [], 2: [], 3: []}
    for f in range(FK):
        jn_fill[1] += proj_granules(qt_h1, wq, bq_t, qT, 2, f)
    for f in range(FK):
        jn_fill[2] += proj_granules(qt_h1, wq, bq_t, qT, 3, f)

    NCH = len(CHUNKS)  # 8 chunks per head pass

    def norm_head(h, ctx_ps, ctxn):
        fq, rq = divmod(h * HD, P)
        sm = nrm.tile([1, 512], f32, tag="sm")
        nc.vector.tensor_copy(sm[:], ctx_ps[HD:HD + 1, :])
        sb = nrm.tile([HD, 512], f32, tag="sb")
        nc.gpsimd.partition_broadcast(sb[:], sm[:])
        rb = nrm.tile([HD, 512], f32, tag="rb")
        rs = nrm.tile([HD, 512], f32, tag="rs")
        nc.vector.reciprocal_approx_accurate(rb[:], sb[:], rs[:])
        nc.vector.tensor_mul(ctxn[fq][rq:rq + HD, :], ctx_ps[0:HD, :], rb[:])

    prev = None
    for j in range(NQ):
        queue = list(j0_fill) if j == 0 else list(jn_fill[j])
        n_early = len(queue)
        if prev is not None:
            queue += outproj_units(prev[0], prev[1])
        pops_per_chunk = 2 if j == 0 else 1
        skip_chunks = 0 if j == 0 else 2  # let the norm chain drain first
        qi = 0
        ctxn = [ctxp.tile([P, 512], bf16, tag=f"ctxn{f}", name=f"ctxn{f}")
                for f in range(FK)]
        ctx_ps_h = {}
        exb = {}           # (h, c) -> ex tile (deferred-consumption buffers)
        pending = []       # ctx chunk work: (h, c)
        done_ctx = {h: 0 for h in range(NHC)}

        def emit_ctx(h, c):
            if h not in ctx_ps_h:
                ctx_ps_h[h] = acc_ps.tile([VW, 512], f32, tag="acc",
                                          name=f"ctxps{h}")
            cp = ctx_ps_h[h]
            ex = exb.pop((h, c))
            for t in range(2):
                kt2 = 2 * c + t
                nc.tensor.matmul(
                    cp[:], vsb[kt2][:, h * VW:(h + 1) * VW],
                    ex[:, t * 512:(t + 1) * 512],
                    start=(kt2 == 0), stop=(kt2 == SK - 1))
            done_ctx[h] += 1
            if done_ctx[h] == NCH:
                norm_head(h, cp, ctxn)

        def vsb_ready_upto(popped):
            vp = max(0, popped - n_vp_before) if j == 0 else 10 ** 9
            return vp // 2 - 1 if j == 0 else 10 ** 9

        for h in range(NHC):
            fq, rq = divmod(h * HD, P)
            qv = qT[fq][rq:rq + HD, j * 512:(j + 1) * 512]
            for c in range(NCH):
                sc = sc_ps.tile([P, 2 * 512], f32, tag="sc")
                for t in range(2):
                    kt2 = 2 * c + t
                    nc.tensor.matmul(
                        sc[:, t * 512:(t + 1) * 512],
                        kT[fq][rq:rq + HD, kt2 * P:(kt2 + 1) * P],
                        qv, start=True, stop=True)
                ex = exp.tile([P, 2 * 512], bf16, tag="ex")
                nc.scalar.activation(ex[:], sc[:], EXP,
                                     scale=1.0 / (HD ** 0.5))
                exb[(h, c)] = ex
                pending.append((h, c))
                gchunk = h * NCH + c
                if gchunk >= skip_chunks or qi < n_early:
                    for _ in range(pops_per_chunk):
                        if qi < len(queue):
                            queue[qi]()
                            qi += 1
                tmax = vsb_ready_upto(qi)
                while pending and 2 * pending[0][1] + 1 <= tmax:
                    emit_ctx(*pending.pop(0))
        while qi < len(queue):
            queue[qi]()
            qi += 1
        while pending:
            emit_ctx(*pending.pop(0))
        prev = (j, ctxn)
    for u in outproj_units(prev[0], prev[1]):
        u()


_CACHE = {}


def _build():
    if "nc" in _CACHE:
        return _CACHE["nc"]
    nc = bacc.Bacc("TRN2", target_bir_lowering=False, debug=False)
    QT = nc.dram_tensor("QT", [D, S], bf16, kind="ExternalInput").ap()
    KT = nc.dram_tensor("KT", [D, S], bf16, kind="ExternalInput").ap()
    VT = nc.dram_tensor("VT", [D, S], bf16, kind="ExternalInput").ap()
    WqT = nc.dram_tensor("WqT", [D, FS], bf16, kind="ExternalInput").ap()
    WkT = nc.dram_tensor("WkT", [D, FS], bf16, kind="ExternalInput").ap()
    WvT = nc.dram_tensor("WvT", [D, FS], bf16, kind="ExternalInput").ap()
    WoT = nc.dram_tensor("WoT", [FS, D], bf16, kind="ExternalInput").ap()
    bq = nc.dram_tensor("bq", [FS, 1], f32, kind="ExternalInput").ap()
    bk = nc.dram_tensor("bk", [FS, 1], f32, kind="ExternalInput").ap()
    bv = nc.dram_tensor("bv", [1, FS], f32, kind="ExternalInput").ap()
    OUTP = nc.dram_tensor("OUTP", [S, D], f32, kind="ExternalOutput").ap()
    with tile.TileContext(nc) as tc, ExitStack() as ctx:
        _emit(ctx, tc, nc, (QT, KT, VT, WqT, WkT, WvT, WoT, bq, bk, bv, OUTP))
    nc.compile()
    _CACHE["nc"] = nc
    return nc


def _in_maps(Q, K, V, Wq, bq, Wk, bk, Wv, bv, Wo, bo):
    bf = ml_dtypes.bfloat16

    def cT(a):  # contiguous bf16 transpose
        return np.ascontiguousarray(np.asarray(a).T).astype(bf)

    QTb = [cT(Q[b]) for b in range(B)]
    KTb = [cT(K[b]) for b in range(B)]
    VTb = [cT(V[b]) for b in range(B)]
    c = np.ascontiguousarray
    maps = []
    for core in range(8):
        b, g = divmod(core, NG)
        sl = slice(g * FS, (g + 1) * FS)
        maps.append({
            "QT": QTb[b], "KT": KTb[b], "VT": VTb[b],
            "WqT": cT(Wq[sl, :]), "WkT": cT(Wk[sl, :]),
            "WvT": cT(Wv[sl, :]), "WoT": cT(Wo[:, sl]),
            "bq": c(np.asarray(bq)[sl].reshape(FS, 1)),
            "bk": c(np.asarray(bk)[sl].reshape(FS, 1)),
            "bv": c(np.asarray(bv)[sl].reshape(1, FS)),
        })
    return maps


def kernel(Q, K, V, Wq, bq, Wk, bk, Wv, bv, Wo, bo):
    nc = _build()
    maps = _in_maps(Q, K, V, Wq, bq, Wk, bk, Wv, bv, Wo, bo)
    res = run_bass_kernel_spmd(nc, maps, core_ids=list(range(8)))
    out = np.empty((B, S, D), np.float32)
    for b in range(B):
        acc = res.results[b * NG]["OUTP"].astype(np.float32)
        for g in range(1, NG):
            acc = acc + res.results[b * NG + g]["OUTP"]
        out[b] = acc + np.asarray(bo, np.float32)[None, :]
    return out



# revision 9
# speedup vs baseline: 1.0335x; 1.0335x over previous
"""Trainium2 Bass kernel for nn_MultiHeadAttention (B=2, S=2048, D=1024, H=16).

Sharding (8 cores): data-parallel over batch (2) x tensor-parallel over
head groups (4 groups of 4 heads). Core c handles batch c//4, heads
4*(c%4) .. 4*(c%4)+3.  Each core computes the full attention for its
heads plus its slice of the output projection; the host sums the 4
partial output projections per batch and adds bo.

On-chip layouts (per core):
  qT      [256 feat over 2 tiles of 128, 2048 seq]
  kz[h]   [128, 2048]: head h's k features on its 64-row band matching
          qT, the other 64 rows ZERO.  Score matmuls then contract over
          the full 128 partitions (measured ~3x faster than K=64
          partial-partition matmuls on TRN2).
  v       [2048 keys, 4*65]      (per head: 64 feats + ones column)
  scoresT [keys, queries] tiles -> exp on the scalar engine with the
          1/sqrt(64) scale fused (max-subtract skipped: softmax is
          shift invariant and scores are O(1) here)
  ctxT    [65, queries] accumulated over key tiles; row 64 = sum of exp
          (from the ones column) -> fast-reciprocal -> broadcast -> mul.
All matmuls run in bf16 with fp32 PSUM accumulation; inputs are cast to
bf16 on the host.  Output partials are written fp16 (halves the output
DMA) and summed f32 on the host.

Input DMAs are issued round-robin across the sync/vector/gpsimd/scalar
queues in dependency-priority waves so the first projection's data is
in flight within ~2us of kernel start.

Schedule: the q/k/v projections are emitted as <=4-matmul "filler
granules" popped between attention chunks so they hide in the scalar
engine (exp) bound attention phase.  Tile dependencies are trace-order
based, so every producer granule pops before its first consumer is
emitted.  ctx matmuls for j==0 are deferred (exp tiles buffered) until
the interleaved v projection has produced the needed v tiles.
"""

import sys

for _p in ("/opt/trn_rl_repo",):
    if _p not in sys.path:
        sys.path.insert(0, _p)

from contextlib import ExitStack

import ml_dtypes
import numpy as np

import concourse.bass as bass
import concourse.tile as tile
from concourse import bacc, mybir
from concourse.bass_utils import run_bass_kernel_spmd

B, S, D, H = 2, 2048, 1024, 16
HD = D // H            # 64 head dim
NG = 4                 # head groups (cores per batch)
NHC = H // NG          # 4 heads per core
FS = NHC * HD          # 256 features per core
P = 128
DK = D // P            # 8 contraction tiles for projections
SK = S // P            # 16 key tiles
NQ = S // 512          # 4 query chunks
FK = FS // P           # 2 feature tiles for qT/ctxT
VW = HD + 1            # v feats + ones column

f32 = mybir.dt.float32
f16 = mybir.dt.float16
bf16 = mybir.dt.bfloat16
EXP = mybir.ActivationFunctionType.Exp
CHUNKS = (2,) * 8   # key tiles per exp chunk (16 total)


def _emit(ctx: ExitStack, tc, nc, io):
    QT, KT, VT, WqT, WkT, WvT, WoT, bq, bk, bv, OUTP = io

    xt = ctx.enter_context(tc.tile_pool(name="xt", bufs=24))
    wp = ctx.enter_context(tc.tile_pool(name="wp", bufs=1))
    per = ctx.enter_context(tc.tile_pool(name="per", bufs=1))
    exp = ctx.enter_context(tc.tile_pool(name="exp", bufs=26))
    nrm = ctx.enter_context(tc.tile_pool(name="nrm", bufs=2))
    ctxp = ctx.enter_context(tc.tile_pool(name="ctxp", bufs=2))
    outp = ctx.enter_context(tc.tile_pool(name="outp", bufs=4))
    sc_ps = ctx.enter_context(tc.tile_pool(name="sc_ps", bufs=2, space="PSUM"))
    acc_ps = ctx.enter_context(tc.tile_pool(name="acc_ps", bufs=4, space="PSUM"))

    # ---- weights / biases (persistent) ----
    wq = [wp.tile([P, FS], bf16, tag=f"wq{k}", name=f"wq{k}") for k in range(DK)]
    wk = [wp.tile([P, FS], bf16, tag=f"wk{k}", name=f"wk{k}") for k in range(DK)]
    wv = [wp.tile([P, FS], bf16, tag=f"wv{k}", name=f"wv{k}") for k in range(DK)]
    wo = [wp.tile([P, D], bf16, tag=f"wo{f}", name=f"wo{f}") for f in range(FK)]
    bq_t = [wp.tile([P, 1], f32, tag=f"bq{f}", name=f"bqt{f}") for f in range(FK)]
    bk_t = [wp.tile([P, 1], f32, tag=f"bk{f}", name=f"bkt{f}") for f in range(FK)]
    bv_t = wp.tile([P, FS], f32, tag="bv")
    ones_t = wp.tile([P, NHC], f32, tag="ones")

    # ---- persistent activations ----
    kz = [per.tile([P, S], bf16, tag=f"kz{h}", name=f"kzs{h}") for h in range(NHC)]
    qT = [per.tile([P, S], bf16, tag=f"qT{f}", name=f"qTs{f}") for f in range(FK)]
    vsb = [per.tile([P, NHC * VW], bf16, tag=f"v{t}", name=f"vs{t}")
           for t in range(SK)]

    # ---- input DMA scheduling: round-robin queues, priority waves ----
    ENG = [nc.sync, nc.gpsimd, nc.scalar]
    _rr = [0]

    def rr():
        e = ENG[_rr[0] % len(ENG)]
        _rr[0] += 1
        return e

    def load_half(src, hf):
        tiles = {}
        for k in range(DK):
            t = xt.tile([P, 1024], bf16, tag="xt", name="xtile")
            rr().dma_start(t[:], src[k * P:(k + 1) * P,
                                     hf * 1024:(hf + 1) * 1024])
            tiles[k] = t
        return tiles

    # memsets all execute on gpsimd; warm_sb first so warmup starts at ~0
    warm_sb = wp.tile([P, 16], f32, tag="warm")
    nc.vector.memset(warm_sb[:], 0.0)

    # wave 1: first k projection's data
    for k in range(DK):
        rr().dma_start(wk[k][:], WkT[k * P:(k + 1) * P, :])
    kt_h0 = load_half(KT, 0)
    for f in range(FK):
        rr().dma_start(bk_t[f][:], bk[f * P:(f + 1) * P, :])
    for h in range(NHC):
        nc.vector.memset(kz[h][:], 0.0)
    nc.vector.memset(ones_t[:], 1.0)

    # HAM pre-warm: ~5us of dependency-free matmuls while the first DMAs
    # land, so real matmuls start at 2.4 GHz instead of 1.2 GHz.
    warm_ps = acc_ps.tile([16, 16], f32, tag="acc", name="warmps")
    for _ in range(100):
        nc.tensor.matmul(warm_ps[:], warm_sb[:, 0:16], warm_sb[:],
                         start=True, stop=True)
    warm_ex = wp.tile([P, 16], bf16, tag="warmex")
    nc.scalar.activation(warm_ex[:], warm_sb[:], EXP, scale=0.125)

    # wave 2: first q projection
    for k in range(DK):
        rr().dma_start(wq[k][:], WqT[k * P:(k + 1) * P, :])
    qt_h0 = load_half(QT, 0)
    for f in range(FK):
        rr().dma_start(bq_t[f][:], bq[f * P:(f + 1) * P, :])
    # waves 3+4 keep the scalar queue free for exp
    ENG[:] = [nc.sync, nc.gpsimd]
    # wave 3: rest of k, first v half, v weights
    kt_h1 = load_half(KT, 1)
    vt_h0 = load_half(VT, 0)
    for k in range(DK):
        rr().dma_start(wv[k][:], WvT[k * P:(k + 1) * P, :])
    rr().dma_start(bv_t[:], bv.to_broadcast((P, FS)))
    # wave 4: the rest
    vt_h1 = load_half(VT, 1)
    qt_h1 = load_half(QT, 1)
    for f in range(FK):
        rr().dma_start(wo[f][:], WoT[f * P:(f + 1) * P, :])

    kt_h = [kt_h0, kt_h1]
    vt_h = [vt_h0, vt_h1]

    # ---- projection writers ----
    def q_write(ps, f, ncol):
        nc.vector.tensor_scalar_add(
            qT[f][:, ncol * 512:(ncol + 1) * 512], ps[:], bq_t[f][:])

    def kz_write(ps, f, ncol):
        c0, c1 = ncol * 512, (ncol + 1) * 512
        nc.vector.tensor_scalar_add(
            kz[2 * f][0:HD, c0:c1], ps[0:HD, :], bk_t[f][0:HD])
        nc.vector.tensor_scalar_add(
            kz[2 * f + 1][HD:P, c0:c1], ps[HD:P, :], bk_t[f][HD:P])

    def proj_cols(src_tiles, w, writer, ncol):
        off = (ncol * 512) % 1024
        for f in range(FK):
            ps = acc_ps.tile([P, 512], f32, tag="acc")
            for k in range(DK):
                nc.tensor.matmul(
                    ps[:],
                    w[k][:, f * P:(f + 1) * P],
                    src_tiles[k][:, off:off + 512],
                    start=(k == 0), stop=(k == DK - 1),
                )
            writer(ps, f, ncol)

    proj_cols(kt_h0, wk, kz_write, 0)
    proj_cols(qt_h0, wq, q_write, 0)

    # ---- filler granules (<=4 matmuls each) ----
    def proj_granules(src_tiles, w, writer, ncol, f):
        off = (ncol * 512) % 1024
        st = {}

        def g0():
            st["ps"] = acc_ps.tile([P, 512], f32, tag="acc", name="pps")
            for k in range(4):
                nc.tensor.matmul(
                    st["ps"][:], w[k][:, f * P:(f + 1) * P],
                    src_tiles[k][:, off:off + 512],
                    start=(k == 0), stop=False)

        def g1():
            for k in range(4, DK):
                nc.tensor.matmul(
                    st["ps"][:], w[k][:, f * P:(f + 1) * P],
                    src_tiles[k][:, off:off + 512],
                    start=False, stop=(k == DK - 1))
            writer(st["ps"], f, ncol)
        return [g0, g1]

    def vproj_granules(t):
        st = {}

        def g0():
            st["ps"] = acc_ps.tile([P, FS], f32, tag="acc", name="vps")
            for k in range(4):
                nc.tensor.matmul(
                    st["ps"][:],
                    vt_h[t * P // 1024][k][:, (t * P) % 1024:(t * P) % 1024 + P],
                    wv[k][:], start=(k == 0), stop=False)

        def g1():
            for k in range(4, DK):
                nc.tensor.matmul(
                    st["ps"][:],
                    vt_h[t * P // 1024][k][:, (t * P) % 1024:(t * P) % 1024 + P],
                    wv[k][:], start=False, stop=(k == DK - 1))
            for h in range(NHC):
                nc.vector.tensor_add(
                    vsb[t][:, h * VW:h * VW + HD],
                    st["ps"][:, h * HD:(h + 1) * HD],
                    bv_t[:, h * HD:(h + 1) * HD])
            nc.vector.tensor_copy(vsb[t][:, HD:NHC * VW:VW], ones_t[:])
        return [g0, g1]

    def outproj_unit(j, ctxn, mt, oc):
        def emit():
            ps = acc_ps.tile([P, 512], f32, tag="acc", name="ops")
            for f in range(FK):
                nc.tensor.matmul(
                    ps[:], ctxn[f][:, mt * P:(mt + 1) * P],
                    wo[f][:, oc * 512:(oc + 1) * 512],
                    start=(f == 0), stop=(f == FK - 1))
            ob = outp.tile([P, 512], f32, tag="ob")
            nc.vector.tensor_copy(ob[:], ps[:])
            # gpsimd DMA casts f32->f16 in flight (halves output traffic)
            nc.gpsimd.dma_start(
                OUTP[j * 512 + mt * P: j * 512 + (mt + 1) * P,
                     oc * 512:(oc + 1) * 512], ob[:])
        return emit

    def outproj_units(j, ctxn):
        return [outproj_unit(j, ctxn, mt, oc)
                for mt in range(4) for oc in range(2)]

    # NOTE: Tile dependencies are trace-order based -- every producer must
    # be EMITTED before its consumer.  All projection fillers therefore pop
    # inside j==0 (before any j>=1 instruction is traced).
    j0_fill = []
    for ncol in (1, 2, 3):
        for f in range(FK):
            j0_fill += proj_granules(kt_h[ncol // 2], wk, kz_write, ncol, f)
    n_vp_before = len(j0_fill)  # 12 kz granules popped first
    for t in range(SK):
        j0_fill += vproj_granules(t)
    for f in range(FK):
        j0_fill += proj_granules(qt_h0, wq, q_write, 1, f)
    jn_fill = {1: [], 2: [], 3: []}
    for f in range(FK):
        jn_fill[1] += proj_granules(qt_h1, wq, q_write, 2, f)
    for f in range(FK):
        jn_fill[2] += proj_granules(qt_h1, wq, q_write, 3, f)

    NCH = len(CHUNKS)  # 8 chunks per head pass

    def norm_head(h, ctx_ps, ctxn):
        fq, rq = divmod(h * HD, P)
        sm = nrm.tile([1, 512], f32, tag="sm")
        nc.vector.tensor_copy(sm[:], ctx_ps[HD:HD + 1, :])
        rb1 = nrm.tile([1, 512], f32, tag="rc")
        nc.vector.reciprocal_approx_fast(rb1[:], sm[:])
        rbb = nrm.tile([HD, 512], f32, tag="rb")
        nc.gpsimd.partition_broadcast(rbb[:], rb1[:])
        nc.vector.tensor_mul(ctxn[fq][rq:rq + HD, :], ctx_ps[0:HD, :], rbb[:])

    prev = None
    for j in range(NQ):
        queue = list(j0_fill) if j == 0 else list(jn_fill[j])
        n_early = len(queue)
        if prev is not None:
            queue += outproj_units(prev[0], prev[1])
        pops_per_chunk = 2 if j == 0 else 1
        skip_chunks = 0 if j == 0 else 2  # let the norm chain drain first
        qi = 0
        ctxn = [ctxp.tile([P, 512], bf16, tag=f"ctxn{f}", name=f"ctxn{f}")
                for f in range(FK)]
        ctx_ps_h = {}
        exb = {}           # (h, c) -> ex tile (deferred-consumption buffers)
        pending = []       # ctx chunk work: (h, c)
        done_ctx = {h: 0 for h in range(NHC)}

        def emit_ctx(h, c):
            if h not in ctx_ps_h:
                ctx_ps_h[h] = acc_ps.tile([VW, 512], f32, tag="acc",
                                          name=f"ctxps{h}")
            cp = ctx_ps_h[h]
            ex = exb.pop((h, c))
            for t in range(2):
                kt2 = 2 * c + t
                nc.tensor.matmul(
                    cp[:], vsb[kt2][:, h * VW:(h + 1) * VW],
                    ex[:, t * 512:(t + 1) * 512],
                    start=(kt2 == 0), stop=(kt2 == SK - 1))
            done_ctx[h] += 1
            if done_ctx[h] == NCH:
                norm_head(h, cp, ctxn)

        def vsb_ready_upto(popped):
            vp = max(0, popped - n_vp_before) if j == 0 else 10 ** 9
            return vp // 2 - 1 if j == 0 else 10 ** 9

        for h in range(NHC):
            qv = qT[h // 2][:, j * 512:(j + 1) * 512]
            for c in range(NCH):
                sc = sc_ps.tile([P, 2 * 512], f32, tag="sc")
                for t in range(2):
                    kt2 = 2 * c + t
                    nc.tensor.matmul(
                        sc[:, t * 512:(t + 1) * 512],
                        kz[h][:, kt2 * P:(kt2 + 1) * P],
                        qv, start=True, stop=True)
                ex = exp.tile([P, 2 * 512], bf16, tag="ex")
                nc.scalar.activation(ex[:], sc[:], EXP,
                                     scale=1.0 / (HD ** 0.5))
                exb[(h, c)] = ex
                pending.append((h, c))
                gchunk = h * NCH + c
                if gchunk >= skip_chunks or qi < n_early:
                    for _ in range(pops_per_chunk):
                        if qi < len(queue):
                            queue[qi]()
                            qi += 1
                tmax = vsb_ready_upto(qi)
                while pending and 2 * pending[0][1] + 1 <= tmax:
                    emit_ctx(*pending.pop(0))
        while qi < len(queue):
            queue[qi]()
            qi += 1
        while pending:
            emit_ctx(*pending.pop(0))
        prev = (j, ctxn)
    for u in outproj_units(prev[0], prev[1]):
        u()


_CACHE = {}


def _build():
    if "nc" in _CACHE:
        return _CACHE["nc"]
    nc = bacc.Bacc("TRN2", target_bir_lowering=False, debug=False)
    QT = nc.dram_tensor("QT", [D, S], bf16, kind="ExternalInput").ap()
    KT = nc.dram_tensor("KT", [D, S], bf16, kind="ExternalInput").ap()
    VT = nc.dram_tensor("VT", [D, S], bf16, kind="ExternalInput").ap()
    WqT = nc.dram_tensor("WqT", [D, FS], bf16, kind="ExternalInput").ap()
    WkT = nc.dram_tensor("WkT", [D, FS], bf16, kind="ExternalInput").ap()
    WvT = nc.dram_tensor("WvT", [D, FS], bf16, kind="ExternalInput").ap()
    WoT = nc.dram_tensor("WoT", [FS, D], bf16, kind="ExternalInput").ap()
    bq = nc.dram_tensor("bq", [FS, 1], f32, kind="ExternalInput").ap()
    bk = nc.dram_tensor("bk", [FS, 1], f32, kind="ExternalInput").ap()
    bv = nc.dram_tensor("bv", [1, FS], f32, kind="ExternalInput").ap()
    OUTP = nc.dram_tensor("OUTP", [S, D], f16, kind="ExternalOutput").ap()
    with tile.TileContext(nc) as tc, ExitStack() as ctx:
        _emit(ctx, tc, nc, (QT, KT, VT, WqT, WkT, WvT, WoT, bq, bk, bv, OUTP))
    nc.compile()
    _CACHE["nc"] = nc
    return nc


def _in_maps(Q, K, V, Wq, bq, Wk, bk, Wv, bv, Wo, bo):
    bf = ml_dtypes.bfloat16

    def cT(a):  # contiguous bf16 transpose
        return np.ascontiguousarray(np.asarray(a).T).astype(bf)

    QTb = [cT(Q[b]) for b in range(B)]
    KTb = [cT(K[b]) for b in range(B)]
    VTb = [cT(V[b]) for b in range(B)]
    c = np.ascontiguousarray
    maps = []
    for core in range(8):
        b, g = divmod(core, NG)
        sl = slice(g * FS, (g + 1) * FS)
        maps.append({
            "QT": QTb[b], "KT": KTb[b], "VT": VTb[b],
            "WqT": cT(Wq[sl, :]), "WkT": cT(Wk[sl, :]),
            "WvT": cT(Wv[sl, :]), "WoT": cT(Wo[:, sl]),
            "bq": c(np.asarray(bq)[sl].reshape(FS, 1)),
            "bk": c(np.asarray(bk)[sl].reshape(FS, 1)),
            "bv": c(np.asarray(bv)[sl].reshape(1, FS)),
        })
    return maps


def kernel(Q, K, V, Wq, bq, Wk, bk, Wv, bv, Wo, bo):
    nc = _build()
    maps = _in_maps(Q, K, V, Wq, bq, Wk, bk, Wv, bv, Wo, bo)
    res = run_bass_kernel_spmd(nc, maps, core_ids=list(range(8)))
    out = np.empty((B, S, D), np.float32)
    for b in range(B):
        acc = res.results[b * NG]["OUTP"].astype(np.float32)
        for g in range(1, NG):
            acc = acc + res.results[b * NG + g]["OUTP"].astype(np.float32)
        out[b] = acc + np.asarray(bo, np.float32)[None, :]
    return out


# revision 15
# speedup vs baseline: 1.0355x; 1.0019x over previous
"""Trainium2 Bass kernel for nn_MultiHeadAttention (B=2, S=2048, D=1024, H=16).

Sharding (8 cores): data-parallel over batch (2) x tensor-parallel over
head groups (4 groups of 4 heads). Core c handles batch c//4, heads
4*(c%4) .. 4*(c%4)+3.  Each core computes the full attention for its
heads plus its slice of the output projection; the host sums the 4
partial output projections per batch and adds bo.

On-chip layouts (per core):
  qT      [256 feat over 2 tiles of 128, 2048 seq]
  kz[h]   [128, 2048]: head h's k features on its 64-row band matching
          qT, the other 64 rows ZERO.  Score matmuls then contract over
          the full 128 partitions (measured ~3x faster than K=64
          partial-partition matmuls on TRN2).
  v       [2048 keys, 4*65]      (per head: 64 feats + ones column)
  scoresT [keys, queries] tiles -> exp on the scalar engine with the
          1/sqrt(64) scale fused (max-subtract skipped: softmax is
          shift invariant and scores are O(1) here)
  ctxT    [65, queries] accumulated over key tiles; row 64 = sum of exp
          (from the ones column) -> fast-reciprocal -> broadcast -> mul.
All matmuls run in bf16 with fp32 PSUM accumulation; inputs are cast to
bf16 on the host.  Output partials are written fp16 (halves the output
DMA) and summed f32 on the host.

Input DMAs are issued round-robin across the sync/vector/gpsimd/scalar
queues in dependency-priority waves so the first projection's data is
in flight within ~2us of kernel start.

Schedule: the q/k/v projections are emitted as <=4-matmul "filler
granules" popped between attention chunks so they hide in the scalar
engine (exp) bound attention phase.  Tile dependencies are trace-order
based, so every producer granule pops before its first consumer is
emitted.  ctx matmuls for j==0 are deferred (exp tiles buffered) until
the interleaved v projection has produced the needed v tiles.
"""

import sys

for _p in ("/opt/trn_rl_repo",):
    if _p not in sys.path:
        sys.path.insert(0, _p)

from contextlib import ExitStack

import ml_dtypes
import numpy as np

import concourse.bass as bass
import concourse.tile as tile
from concourse import bacc, mybir
from concourse.bass_utils import run_bass_kernel_spmd

B, S, D, H = 2, 2048, 1024, 16
HD = D // H            # 64 head dim
NG = 4                 # head groups (cores per batch)
NHC = H // NG          # 4 heads per core
FS = NHC * HD          # 256 features per core
P = 128
DK = D // P            # 8 contraction tiles for projections
SK = S // P            # 16 key tiles
NQ = S // 512          # 4 query chunks
FK = FS // P           # 2 feature tiles for qT/ctxT
VW = HD + 1            # v feats + ones column

f32 = mybir.dt.float32
f16 = mybir.dt.float16
bf16 = mybir.dt.bfloat16
EXP = mybir.ActivationFunctionType.Exp
CHUNKS = (2,) * 8   # key tiles per exp chunk (16 total)


def _emit(ctx: ExitStack, tc, nc, io):
    QT, KT, VT, WqT, WkT, WvT, WoT, bq, bk, bv, OUTP = io

    xt = ctx.enter_context(tc.tile_pool(name="xt", bufs=24))
    wp = ctx.enter_context(tc.tile_pool(name="wp", bufs=1))
    per = ctx.enter_context(tc.tile_pool(name="per", bufs=1))
    exp = ctx.enter_context(tc.tile_pool(name="exp", bufs=26))
    nrm = ctx.enter_context(tc.tile_pool(name="nrm", bufs=2))
    ctxp = ctx.enter_context(tc.tile_pool(name="ctxp", bufs=2))
    outp = ctx.enter_context(tc.tile_pool(name="outp", bufs=4))
    sc_ps = ctx.enter_context(tc.tile_pool(name="sc_ps", bufs=2, space="PSUM"))
    # prj_ps: projection pairs + outproj units (popped sequentially, never
    # concurrent).  acc_ps: ctx accumulators only.
    prj_ps = ctx.enter_context(tc.tile_pool(name="prj_ps", bufs=2, space="PSUM"))
    acc_ps = ctx.enter_context(tc.tile_pool(name="acc_ps", bufs=2, space="PSUM"))

    # ---- weights / biases (persistent) ----
    wq = [wp.tile([P, FS], bf16, tag=f"wq{k}", name=f"wq{k}") for k in range(DK)]
    wk = [wp.tile([P, FS], bf16, tag=f"wk{k}", name=f"wk{k}") for k in range(DK)]
    wv = [wp.tile([P, FS], bf16, tag=f"wv{k}", name=f"wv{k}") for k in range(DK)]
    wo = [wp.tile([P, D], bf16, tag=f"wo{f}", name=f"wo{f}") for f in range(FK)]
    bq_t = [wp.tile([P, 1], f32, tag=f"bq{f}", name=f"bqt{f}") for f in range(FK)]
    bk_t = [wp.tile([P, 1], f32, tag=f"bk{f}", name=f"bkt{f}") for f in range(FK)]
    bv_t = wp.tile([P, FS], f32, tag="bv")
    ones_t = wp.tile([P, NHC], f32, tag="ones")

    # ---- persistent activations ----
    kz = [per.tile([P, S], bf16, tag=f"kz{h}", name=f"kzs{h}") for h in range(NHC)]
    qT = [per.tile([P, S], bf16, tag=f"qT{f}", name=f"qTs{f}") for f in range(FK)]
    vsb = [per.tile([P, NHC * VW], bf16, tag=f"v{t}", name=f"vs{t}")
           for t in range(SK)]

    # ---- input DMA scheduling: round-robin queues, priority waves ----
    ENG = [nc.sync, nc.gpsimd, nc.scalar]
    _rr = [0]

    def rr():
        e = ENG[_rr[0] % len(ENG)]
        _rr[0] += 1
        return e

    def load_half(src, hf):
        tiles = {}
        for k in range(DK):
            t = xt.tile([P, 1024], bf16, tag="xt", name="xtile")
            rr().dma_start(t[:], src[k * P:(k + 1) * P,
                                     hf * 1024:(hf + 1) * 1024])
            tiles[k] = t
        return tiles

    # memsets all execute on gpsimd; warm_sb first so warmup starts at ~0
    warm_sb = wp.tile([P, 16], f32, tag="warm")
    nc.vector.memset(warm_sb[:], 0.0)

    # wave 1: first k projection's data
    for k in range(DK):
        rr().dma_start(wk[k][:], WkT[k * P:(k + 1) * P, :])
    kt_h0 = load_half(KT, 0)
    for f in range(FK):
        rr().dma_start(bk_t[f][:], bk[f * P:(f + 1) * P, :])
    for h in range(NHC):
        nc.vector.memset(kz[h][:], 0.0)
    nc.vector.memset(ones_t[:], 1.0)

    # HAM pre-warm: ~5us of dependency-free matmuls while the first DMAs
    # land, so real matmuls start at 2.4 GHz instead of 1.2 GHz.
    warm_ps = acc_ps.tile([16, 16], f32, tag="acc", name="warmps")
    for _ in range(100):
        nc.tensor.matmul(warm_ps[:], warm_sb[:, 0:16], warm_sb[:],
                         start=True, stop=True)
    warm_ex = wp.tile([P, 16], bf16, tag="warmex")
    nc.scalar.activation(warm_ex[:], warm_sb[:], EXP, scale=0.125)

    # wave 2: first q projection
    for k in range(DK):
        rr().dma_start(wq[k][:], WqT[k * P:(k + 1) * P, :])
    qt_h0 = load_half(QT, 0)
    for f in range(FK):
        rr().dma_start(bq_t[f][:], bq[f * P:(f + 1) * P, :])
    # waves 3+4 keep the scalar queue free for exp
    ENG[:] = [nc.sync, nc.gpsimd]
    # wave 3: rest of k, first v half, v weights
    kt_h1 = load_half(KT, 1)
    vt_h0 = load_half(VT, 0)
    for k in range(DK):
        rr().dma_start(wv[k][:], WvT[k * P:(k + 1) * P, :])
    rr().dma_start(bv_t[:], bv.to_broadcast((P, FS)))
    # wave 4: the rest
    vt_h1 = load_half(VT, 1)
    qt_h1 = load_half(QT, 1)
    for f in range(FK):
        rr().dma_start(wo[f][:], WoT[f * P:(f + 1) * P, :])

    kt_h = [kt_h0, kt_h1]
    vt_h = [vt_h0, vt_h1]

    # ---- projection writers ----
    def q_write(pss, ncol):
        for f in range(FK):
            nc.vector.tensor_scalar_add(
                qT[f][:, ncol * 512:(ncol + 1) * 512], pss[f][:], bq_t[f][:])

    def kz_write(pss, ncol):
        c0, c1 = ncol * 512, (ncol + 1) * 512
        for f in range(FK):
            nc.vector.tensor_scalar_add(
                kz[2 * f][0:HD, c0:c1], pss[f][0:HD, :], bk_t[f][0:HD])
            nc.vector.tensor_scalar_add(
                kz[2 * f + 1][HD:P, c0:c1], pss[f][HD:P, :], bk_t[f][HD:P])

    # Both feature tiles accumulate against the SAME moving x chunk
    # (fixed-moving matmuls measure ~70ns cheaper than rotating-moving).
    def proj_cols(src_tiles, w, writer, ncol):
        off = (ncol * 512) % 1024
        pss = [prj_ps.tile([P, 512], f32, tag="prj", name=f"pc{f}")
               for f in range(FK)]
        for k in range(DK):
            x = src_tiles[k][:, off:off + 512]
            for f in range(FK):
                nc.tensor.matmul(pss[f][:], w[k][:, f * P:(f + 1) * P], x,
                                 start=(k == 0), stop=(k == DK - 1))
        writer(pss, ncol)

    proj_cols(kt_h0, wk, kz_write, 0)
    proj_cols(qt_h0, wq, q_write, 0)

    # ---- filler granules (2 matmuls each, fixed moving x) ----
    def proj_granules(src_tiles, w, writer, ncol):
        off = (ncol * 512) % 1024
        st = {}
        gs = []

        def gk(k):
            def g():
                if k == 0:
                    st["ps"] = [prj_ps.tile([P, 512], f32, tag="prj",
                                            name=f"pp{f}") for f in range(FK)]
                x = src_tiles[k][:, off:off + 512]
                for f in range(FK):
                    nc.tensor.matmul(st["ps"][f][:],
                                     w[k][:, f * P:(f + 1) * P], x,
                                     start=(k == 0), stop=(k == DK - 1))
            return g
        gs = [gk(k) for k in range(DK)]
        gs.append(lambda: writer(st["ps"], ncol))
        return gs

    # v projection in seq-tile pairs: both tiles stream the same wv[k]
    def vproj_granules(tp):
        t0, t1 = 2 * tp, 2 * tp + 1
        st = {}

        def vslice(t, k):
            return vt_h[t * P // 1024][k][:, (t * P) % 1024:(t * P) % 1024 + P]

        def gk(k):
            def g():
                if k == 0:
                    st["ps"] = [prj_ps.tile([P, FS], f32, tag="prj",
                                            name=f"vp{i}") for i in range(2)]
                for i, t in enumerate((t0, t1)):
                    nc.tensor.matmul(st["ps"][i][:], vslice(t, k), wv[k][:],
                                     start=(k == 0), stop=(k == DK - 1))
            return g

        def wr():
            for i, t in enumerate((t0, t1)):
                for h in range(NHC):
                    nc.vector.tensor_add(
                        vsb[t][:, h * VW:h * VW + HD],
                        st["ps"][i][:, h * HD:(h + 1) * HD],
                        bv_t[:, h * HD:(h + 1) * HD])
                nc.vector.tensor_copy(vsb[t][:, HD:NHC * VW:VW], ones_t[:])
        return [gk(k) for k in range(DK)] + [wr]

    def outproj_unit(j, ctxn, mt, oc, idx, last):
        def emit():
            ps = prj_ps.tile([P, 512], f32, tag="prj", name="ops")
            for f in range(FK):
                nc.tensor.matmul(
                    ps[:], ctxn[f][:, mt * P:(mt + 1) * P],
                    wo[f][:, oc * 512:(oc + 1) * 512],
                    start=(f == 0), stop=(f == FK - 1))
            # in the tail, spread copies over scalar+vector and DMA issues
            # over gpsimd+sync to shorten the serial epilogue.  Only gpsimd
            # DMAs can cast f32->f16, so the sync path casts in the copy.
            dst = OUTP[j * 512 + mt * P: j * 512 + (mt + 1) * P,
                       oc * 512:(oc + 1) * 512]
            if last and idx % 2 == 1:
                ob = outp.tile([P, 512], f16, tag="obh", name="obh")
                nc.vector.tensor_copy(ob[:], ps[:])
                nc.sync.dma_start(dst, ob[:])
            else:
                ob = outp.tile([P, 512], f32, tag="ob")
                if last:
                    nc.scalar.copy(ob[:], ps[:])
                else:
                    nc.vector.tensor_copy(ob[:], ps[:])
                nc.gpsimd.dma_start(dst, ob[:])
        return emit

    def outproj_units(j, ctxn, last=False):
        return [outproj_unit(j, ctxn, mt, oc, mt * 2 + oc, last)
                for mt in range(4) for oc in range(2)]

    # NOTE: Tile dependencies are trace-order based -- every producer must
    # be EMITTED before its consumer.  All projection fillers therefore pop
    # inside j==0 (before any j>=1 instruction is traced).
    j0_fill = []
    for ncol in (1, 2, 3):
        j0_fill += proj_granules(kt_h[ncol // 2], wk, kz_write, ncol)
    n_vp_before = len(j0_fill)  # 27 kz granules popped first
    GPP = DK + 1  # granules per v seq-tile pair
    for tp in range(SK // 2):
        j0_fill += vproj_granules(tp)
    j0_fill += proj_granules(qt_h0, wq, q_write, 1)
    jn_fill = {1: [], 2: [], 3: []}
    jn_fill[1] += proj_granules(qt_h1, wq, q_write, 2)
    jn_fill[2] += proj_granules(qt_h1, wq, q_write, 3)

    NCH = len(CHUNKS)  # 8 chunks per head pass

    def norm_head(h, ctx_ps, ctxn):
        fq, rq = divmod(h * HD, P)
        sm = nrm.tile([1, 512], f32, tag="sm")
        nc.vector.tensor_copy(sm[:], ctx_ps[HD:HD + 1, :])
        rb1 = nrm.tile([1, 512], f32, tag="rc")
        nc.vector.reciprocal_approx_fast(rb1[:], sm[:])
        rbb = nrm.tile([HD, 512], f32, tag="rb")
        nc.gpsimd.partition_broadcast(rbb[:], rb1[:])
        nc.vector.tensor_mul(ctxn[fq][rq:rq + HD, :], ctx_ps[0:HD, :], rbb[:])

    prev = None
    for j in range(NQ):
        queue = list(j0_fill) if j == 0 else list(jn_fill[j])
        n_early = len(queue)
        if prev is not None:
            queue += outproj_units(prev[0], prev[1])
        pops_per_chunk = 4 if j == 0 else 1
        skip_chunks = 0 if j == 0 else 2  # let the norm chain drain first
        qi = 0
        ctxn = [ctxp.tile([P, 512], bf16, tag=f"ctxn{f}", name=f"ctxn{f}")
                for f in range(FK)]
        ctx_ps_h = {}
        exb = {}           # (h, c) -> ex tile (deferred-consumption buffers)
        pending = []       # ctx chunk work: (h, c)
        done_ctx = {h: 0 for h in range(NHC)}

        def emit_ctx(h, c):
            if h not in ctx_ps_h:
                ctx_ps_h[h] = acc_ps.tile([VW, 512], f32, tag="acc",
                                          name=f"ctxps{h}")
            cp = ctx_ps_h[h]
            ex = exb.pop((h, c))
            for t in range(2):
                kt2 = 2 * c + t
                nc.tensor.matmul(
                    cp[:], vsb[kt2][:, h * VW:(h + 1) * VW],
                    ex[:, t * 512:(t + 1) * 512],
                    start=(kt2 == 0), stop=(kt2 == SK - 1))
            done_ctx[h] += 1
            if done_ctx[h] == NCH:
                norm_head(h, cp, ctxn)

        def vsb_ready_upto(popped):
            if j != 0:
                return 10 ** 9
            vp = max(0, popped - n_vp_before)
            return 2 * (vp // GPP) - 1  # v pairs completed -> last tile idx

        for h in range(NHC):
            qv = qT[h // 2][:, j * 512:(j + 1) * 512]
            for c2 in range(NCH // 2):
                if j == 0:
                    # chunk 2*c2+1 reads kz cols of ncol c2: its writer is
                    # queue index 9*c2-1.  Force-pop to keep emission order
                    # = dependency order (Tile deps are trace-order).
                    while qi < min(9 * c2, len(queue)):
                        queue[qi]()
                        qi += 1
                # score burst: 4 matmuls sharing the same moving qv
                scs = []
                for c in (2 * c2, 2 * c2 + 1):
                    sc = sc_ps.tile([P, 2 * 512], f32, tag="sc")
                    for t in range(2):
                        kt2 = 2 * c + t
                        nc.tensor.matmul(
                            sc[:, t * 512:(t + 1) * 512],
                            kz[h][:, kt2 * P:(kt2 + 1) * P],
                            qv, start=True, stop=True)
                    scs.append(sc)
                for i, c in enumerate((2 * c2, 2 * c2 + 1)):
                    ex = exp.tile([P, 2 * 512], bf16, tag="ex")
                    nc.scalar.activation(ex[:], scs[i][:], EXP,
                                         scale=1.0 / (HD ** 0.5))
                    exb[(h, c)] = ex
                    pending.append((h, c))
                    gchunk = h * NCH + c
                    if gchunk >= skip_chunks or qi < n_early:
                        for _ in range(pops_per_chunk):
                            if qi < len(queue):
                                queue[qi]()
                                qi += 1
                    tmax = vsb_ready_upto(qi)
                    while pending and 2 * pending[0][1] + 1 <= tmax:
                        emit_ctx(*pending.pop(0))
        while qi < len(queue):
            queue[qi]()
            qi += 1
        while pending:
            emit_ctx(*pending.pop(0))
        prev = (j, ctxn)
    for u in outproj_units(prev[0], prev[1], last=True):
        u()


_CACHE = {}


def _build():
    if "nc" in _CACHE:
        return _CACHE["nc"]
    nc = bacc.Bacc("TRN2", target_bir_lowering=False, debug=False)
    QT = nc.dram_tensor("QT", [D, S], bf16, kind="ExternalInput").ap()
    KT = nc.dram_tensor("KT", [D, S], bf16, kind="ExternalInput").ap()
    VT = nc.dram_tensor("VT", [D, S], bf16, kind="ExternalInput").ap()
    WqT = nc.dram_tensor("WqT", [D, FS], bf16, kind="ExternalInput").ap()
    WkT = nc.dram_tensor("WkT", [D, FS], bf16, kind="ExternalInput").ap()
    WvT = nc.dram_tensor("WvT", [D, FS], bf16, kind="ExternalInput").ap()
    WoT = nc.dram_tensor("WoT", [FS, D], bf16, kind="ExternalInput").ap()
    bq = nc.dram_tensor("bq", [FS, 1], f32, kind="ExternalInput").ap()
    bk = nc.dram_tensor("bk", [FS, 1], f32, kind="ExternalInput").ap()
    bv = nc.dram_tensor("bv", [1, FS], f32, kind="ExternalInput").ap()
    OUTP = nc.dram_tensor("OUTP", [S, D], f16, kind="ExternalOutput").ap()
    with tile.TileContext(nc) as tc, ExitStack() as ctx:
        _emit(ctx, tc, nc, (QT, KT, VT, WqT, WkT, WvT, WoT, bq, bk, bv, OUTP))
    nc.compile()
    _CACHE["nc"] = nc
    return nc


def _in_maps(Q, K, V, Wq, bq, Wk, bk, Wv, bv, Wo, bo):
    bf = ml_dtypes.bfloat16

    def cT(a):  # contiguous bf16 transpose
        return np.ascontiguousarray(np.asarray(a).T).astype(bf)

    QTb = [cT(Q[b]) for b in range(B)]
    KTb = [cT(K[b]) for b in range(B)]
    VTb = [cT(V[b]) for b in range(B)]
    c = np.ascontiguousarray
    maps = []
    for core in range(8):
        b, g = divmod(core, NG)
        sl = slice(g * FS, (g + 1) * FS)
        maps.append({
            "QT": QTb[b], "KT": KTb[b], "VT": VTb[b],
            "WqT": cT(Wq[sl, :]), "WkT": cT(Wk[sl, :]),
            "WvT": cT(Wv[sl, :]), "WoT": cT(Wo[:, sl]),
            "bq": c(np.asarray(bq)[sl].reshape(FS, 1)),
            "bk": c(np.asarray(bk)[sl].reshape(FS, 1)),
            "bv": c(np.asarray(bv)[sl].reshape(1, FS)),
        })
    return maps


def kernel(Q, K, V, Wq, bq, Wk, bk, Wv, bv, Wo, bo):
    nc = _build()
    maps = _in_maps(Q, K, V, Wq, bq, Wk, bk, Wv, bv, Wo, bo)
    res = run_bass_kernel_spmd(nc, maps, core_ids=list(range(8)))
    out = np.empty((B, S, D), np.float32)
    for b in range(B):
        acc = res.results[b * NG]["OUTP"].astype(np.float32)
        for g in range(1, NG):
            acc = acc + res.results[b * NG + g]["OUTP"].astype(np.float32)
        out[b] = acc + np.asarray(bo, np.float32)[None, :]
    return out


# revision 25
# speedup vs baseline: 1.0620x; 1.0255x over previous
"""Trainium2 Bass kernel for nn_MultiHeadAttention (B=2, S=2048, D=1024, H=16).

Sharding (8 cores): data-parallel over batch (2) x tensor-parallel over
head groups (4 groups of 4 heads). Core c handles batch c//4, heads
4*(c%4) .. 4*(c%4)+3.  Each core computes the full attention for its
heads plus its slice of the output projection; the host sums the 4
partial output projections per batch and adds bo.

On-chip layouts (per core):
  qT      [256 feat over 2 tiles of 128, 2048 seq]
  kz[h]   [128, 2048]: head h's k features on its 64-row band matching
          qT, the other 64 rows ZERO.  Score matmuls then contract over
          the full 128 partitions (measured ~3x faster than K=64
          partial-partition matmuls on TRN2).
  v       [2048 keys, 4*65]      (per head: 64 feats + ones column)
  scoresT [keys, queries] tiles -> exp on the scalar engine with the
          1/sqrt(64) scale fused (max-subtract skipped: softmax is
          shift invariant and scores are O(1) here)
  ctxT    [65, queries] accumulated over key tiles; row 64 = sum of exp
          (from the ones column) -> fast-reciprocal -> broadcast -> mul.
All matmuls run in bf16 with fp32 PSUM accumulation; inputs are cast to
bf16 on the host.  Output partials are written fp16 (halves the output
DMA) and summed f32 on the host.

Input DMAs are issued round-robin across the sync/vector/gpsimd/scalar
queues in dependency-priority waves so the first projection's data is
in flight within ~2us of kernel start.

Schedule: the q/k/v projections are emitted as <=4-matmul "filler
granules" popped between attention chunks so they hide in the scalar
engine (exp) bound attention phase.  Tile dependencies are trace-order
based, so every producer granule pops before its first consumer is
emitted.  ctx matmuls for j==0 are deferred (exp tiles buffered) until
the interleaved v projection has produced the needed v tiles.
"""

import sys

for _p in ("/opt/trn_rl_repo",):
    if _p not in sys.path:
        sys.path.insert(0, _p)

from contextlib import ExitStack

import ml_dtypes
import numpy as np

import concourse.bass as bass
import concourse.tile as tile
from concourse import bacc, mybir
from concourse.bass_utils import run_bass_kernel_spmd

B, S, D, H = 2, 2048, 1024, 16
HD = D // H            # 64 head dim
NG = 4                 # head groups (cores per batch)
NHC = H // NG          # 4 heads per core
FS = NHC * HD          # 256 features per core
P = 128
DK = D // P            # 8 contraction tiles for projections
SK = S // P            # 16 key tiles
NQ = S // 512          # 4 query chunks
FK = FS // P           # 2 feature tiles for qT/ctxT
VW = HD + 1            # v feats + ones column

f32 = mybir.dt.float32
f16 = mybir.dt.float16
bf16 = mybir.dt.bfloat16
f8 = mybir.dt.float8e4
EXP = mybir.ActivationFunctionType.Exp
CHUNKS = (2,) * 8   # key tiles per exp chunk (16 total)
WS = 16.0           # fp8 weight pre-scale (keeps Wq/Wk out of subnormals)


def _emit(ctx: ExitStack, tc, nc, io):
    QT, KT, VT, WqT, WkT, WvT, WoT, bq, bk, bv, OUTP = io

    xt = ctx.enter_context(tc.tile_pool(name="xt", bufs=24))
    wp = ctx.enter_context(tc.tile_pool(name="wp", bufs=1))
    per = ctx.enter_context(tc.tile_pool(name="per", bufs=1))
    exp = ctx.enter_context(tc.tile_pool(name="exp", bufs=26))
    nrm = ctx.enter_context(tc.tile_pool(name="nrm", bufs=2))
    ctxp = ctx.enter_context(tc.tile_pool(name="ctxp", bufs=2))
    outp = ctx.enter_context(tc.tile_pool(name="outp", bufs=4))
    sc_ps = ctx.enter_context(tc.tile_pool(name="sc_ps", bufs=2, space="PSUM"))
    # prj_ps: projection pairs + outproj units (popped sequentially, never
    # concurrent).  acc_ps: ctx accumulators only.
    prj_ps = ctx.enter_context(tc.tile_pool(name="prj_ps", bufs=2, space="PSUM"))
    acc_ps = ctx.enter_context(tc.tile_pool(name="acc_ps", bufs=2, space="PSUM"))

    # ---- weights / biases (persistent); q/k path is fp8 ----
    wq = [wp.tile([P, FS], f8, tag=f"wq{k}", name=f"wq{k}") for k in range(DK)]
    wk = [wp.tile([P, FS], f8, tag=f"wk{k}", name=f"wk{k}") for k in range(DK)]
    wv = [wp.tile([P, FS], bf16, tag=f"wv{k}", name=f"wv{k}") for k in range(DK)]
    wo = [wp.tile([P, D], bf16, tag=f"wo{f}", name=f"wo{f}") for f in range(FK)]
    bq_t = [wp.tile([P, 1], f32, tag=f"bq{f}", name=f"bqt{f}") for f in range(FK)]
    bk_t = [wp.tile([P, 1], f32, tag=f"bk{f}", name=f"bkt{f}") for f in range(FK)]
    bv_t = wp.tile([P, FS], f32, tag="bv")
    ones_t = wp.tile([P, NHC], f32, tag="ones")

    # ---- persistent activations ----
    kz = [per.tile([P, S], bf16, tag=f"kz{h}", name=f"kzs{h}") for h in range(NHC)]
    qT = [per.tile([P, S], bf16, tag=f"qT{f}", name=f"qTs{f}") for f in range(FK)]
    vsb = [per.tile([P, NHC * VW], bf16, tag=f"v{t}", name=f"vs{t}")
           for t in range(SK)]

    # ---- input DMA scheduling: round-robin queues, priority waves ----
    ENG = [nc.sync, nc.gpsimd, nc.scalar]
    _rr = [0]

    def rr():
        e = ENG[_rr[0] % len(ENG)]
        _rr[0] += 1
        return e

    def load_half(src, hf, dt=bf16):
        tiles = {}
        for k in range(DK):
            t = xt.tile([P, 1024], dt, tag="xt", name="xtile")
            rr().dma_start(t[:], src[k * P:(k + 1) * P,
                                     hf * 1024:(hf + 1) * 1024])
            tiles[k] = t
        return tiles

    # memsets all execute on gpsimd; warm_sb first so warmup starts at ~0
    warm_sb = wp.tile([P, 16], f32, tag="warm")
    nc.vector.memset(warm_sb[:], 0.0)

    # wave 1: first k projection's data
    for k in range(DK):
        rr().dma_start(wk[k][:], WkT[k * P:(k + 1) * P, :])
    kt_h0 = load_half(KT, 0, f8)
    for f in range(FK):
        rr().dma_start(bk_t[f][:], bk[f * P:(f + 1) * P, :])
    for h in range(NHC):
        nc.vector.memset(kz[h][:], 0.0)
    nc.vector.memset(ones_t[:], 1.0)

    # HAM pre-warm: ~5us of dependency-free matmuls while the first DMAs
    # land, so real matmuls start at 2.4 GHz instead of 1.2 GHz.
    warm_ps = acc_ps.tile([16, 16], f32, tag="acc", name="warmps")
    for _ in range(100):
        nc.tensor.matmul(warm_ps[:], warm_sb[:, 0:16], warm_sb[:],
                         start=True, stop=True)
    warm_ex = wp.tile([P, 16], bf16, tag="warmex")
    nc.scalar.activation(warm_ex[:], warm_sb[:], EXP, scale=0.125)

    # wave 2: first q projection
    for k in range(DK):
        rr().dma_start(wq[k][:], WqT[k * P:(k + 1) * P, :])
    qt_h0 = load_half(QT, 0, f8)
    for f in range(FK):
        rr().dma_start(bq_t[f][:], bq[f * P:(f + 1) * P, :])
    # waves 3+4 keep the scalar queue free for exp
    ENG[:] = [nc.sync, nc.gpsimd]
    # wave 3: rest of k, first v half, v weights
    kt_h1 = load_half(KT, 1, f8)
    vt_h0 = load_half(VT, 0)
    for k in range(DK):
        rr().dma_start(wv[k][:], WvT[k * P:(k + 1) * P, :])
    rr().dma_start(bv_t[:], bv.to_broadcast((P, FS)))
    # wave 4: the rest
    vt_h1 = load_half(VT, 1)
    qt_h1 = load_half(QT, 1, f8)
    for f in range(FK):
        rr().dma_start(wo[f][:], WoT[f * P:(f + 1) * P, :])

    kt_h = [kt_h0, kt_h1]
    vt_h = [vt_h0, vt_h1]

    # ---- projection writers (1/WS undoes the fp8 weight pre-scale) ----
    MUL, ADD = mybir.AluOpType.mult, mybir.AluOpType.add

    def q_write(pss, ncol):
        for f in range(FK):
            nc.vector.tensor_scalar(
                qT[f][:, ncol * 512:(ncol + 1) * 512], pss[f][:],
                1.0 / WS, bq_t[f][:], MUL, ADD)

    def kz_write(pss, ncol):
        c0, c1 = ncol * 512, (ncol + 1) * 512
        for f in range(FK):
            nc.vector.tensor_scalar(
                kz[2 * f][0:HD, c0:c1], pss[f][0:HD, :],
                1.0 / WS, bk_t[f][0:HD], MUL, ADD)
            nc.vector.tensor_scalar(
                kz[2 * f + 1][HD:P, c0:c1], pss[f][HD:P, :],
                1.0 / WS, bk_t[f][HD:P], MUL, ADD)

    # Both feature tiles accumulate against the SAME moving x chunk
    # (fixed-moving matmuls measure ~70ns cheaper than rotating-moving).
    def proj_cols(src_tiles, w, writer, ncol):
        off = (ncol * 512) % 1024
        pss = [prj_ps.tile([P, 512], f32, tag="prj", name=f"pc{f}")
               for f in range(FK)]
        for k in range(DK):
            x = src_tiles[k][:, off:off + 512]
            for f in range(FK):
                nc.tensor.matmul(pss[f][:], w[k][:, f * P:(f + 1) * P], x,
                                 start=(k == 0), stop=(k == DK - 1))
        writer(pss, ncol)

    proj_cols(kt_h0, wk, kz_write, 0)
    proj_cols(qt_h0, wq, q_write, 0)

    # ---- filler granules (2 matmuls each, fixed moving x) ----
    def proj_granules(src_tiles, w, writer, ncol):
        off = (ncol * 512) % 1024
        st = {}
        gs = []

        def gk(k):
            def g():
                if k == 0:
                    st["ps"] = [prj_ps.tile([P, 512], f32, tag="prj",
                                            name=f"pp{f}") for f in range(FK)]
                x = src_tiles[k][:, off:off + 512]
                for f in range(FK):
                    nc.tensor.matmul(st["ps"][f][:],
                                     w[k][:, f * P:(f + 1) * P], x,
                                     start=(k == 0), stop=(k == DK - 1))
            return g
        gs = [gk(k) for k in range(DK)]
        gs.append(lambda: writer(st["ps"], ncol))
        return gs

    # v projection in seq-tile pairs: both tiles stream the same wv[k]
    def vproj_granules(tp):
        t0, t1 = 2 * tp, 2 * tp + 1
        st = {}

        def vslice(t, k):
            return vt_h[t * P // 1024][k][:, (t * P) % 1024:(t * P) % 1024 + P]

        def gk(k):
            def g():
                if k == 0:
                    st["ps"] = [prj_ps.tile([P, FS], f32, tag="prj",
                                            name=f"vp{i}") for i in range(2)]
                for i, t in enumerate((t0, t1)):
                    nc.tensor.matmul(st["ps"][i][:], vslice(t, k), wv[k][:],
                                     start=(k == 0), stop=(k == DK - 1))
            return g

        def wr():
            for i, t in enumerate((t0, t1)):
                for h in range(NHC):
                    nc.vector.tensor_add(
                        vsb[t][:, h * VW:h * VW + HD],
                        st["ps"][i][:, h * HD:(h + 1) * HD],
                        bv_t[:, h * HD:(h + 1) * HD])
                nc.vector.tensor_copy(vsb[t][:, HD:NHC * VW:VW], ones_t[:])
        return [gk(k) for k in range(DK)] + [wr]

    def outproj_unit(j, ctxn, mt, oc, idx, last):
        def emit():
            ps = prj_ps.tile([P, 512], f32, tag="prj", name="ops")
            for f in range(FK):
                nc.tensor.matmul(
                    ps[:], ctxn[f][:, mt * P:(mt + 1) * P],
                    wo[f][:, oc * 512:(oc + 1) * 512],
                    start=(f == 0), stop=(f == FK - 1))
            # in the tail, spread copies over scalar+vector and DMA issues
            # over gpsimd+sync to shorten the serial epilogue.  Only gpsimd
            # DMAs can cast f32->f16, so the sync path casts in the copy.
            dst = OUTP[j * 512 + mt * P: j * 512 + (mt + 1) * P,
                       oc * 512:(oc + 1) * 512]
            if last and idx % 2 == 1:
                ob = outp.tile([P, 512], f16, tag="obh", name="obh")
                nc.vector.tensor_copy(ob[:], ps[:])
                (nc.sync if idx % 4 == 1 else nc.scalar).dma_start(dst, ob[:])
            else:
                ob = outp.tile([P, 512], f32, tag="ob")
                if last:
                    nc.scalar.copy(ob[:], ps[:])
                else:
                    nc.vector.tensor_copy(ob[:], ps[:])
                nc.gpsimd.dma_start(dst, ob[:])
        return emit

    def outproj_units(j, ctxn, last=False):
        return [outproj_unit(j, ctxn, mt, oc, mt * 2 + oc, last)
                for mt in range(4) for oc in range(2)]

    # NOTE: Tile dependencies are trace-order based -- every producer must
    # be EMITTED before its consumer.  All projection fillers therefore pop
    # inside j==0 (before any j>=1 instruction is traced).
    j0_fill = []
    for ncol in (1, 2, 3):
        j0_fill += proj_granules(kt_h[ncol // 2], wk, kz_write, ncol)
    n_vp_before = len(j0_fill)  # 27 kz granules popped first
    GPP = DK + 1  # granules per v seq-tile pair
    for tp in range(SK // 2):
        j0_fill += vproj_granules(tp)
    j0_fill += proj_granules(qt_h0, wq, q_write, 1)
    jn_fill = {1: [], 2: [], 3: []}
    jn_fill[1] += proj_granules(qt_h1, wq, q_write, 2)
    jn_fill[2] += proj_granules(qt_h1, wq, q_write, 3)

    NCH = len(CHUNKS)  # 8 chunks per head pass

    def norm_head(h, ctx_ps, ctxn):
        fq, rq = divmod(h * HD, P)
        sm = nrm.tile([1, 512], f32, tag="sm")
        nc.vector.tensor_copy(sm[:], ctx_ps[HD:HD + 1, :])
        rb1 = nrm.tile([1, 512], f32, tag="rc")
        nc.vector.reciprocal_approx_fast(rb1[:], sm[:])
        rbb = nrm.tile([HD, 512], f32, tag="rb")
        nc.gpsimd.partition_broadcast(rbb[:], rb1[:])
        nc.vector.tensor_mul(ctxn[fq][rq:rq + HD, :], ctx_ps[0:HD, :], rbb[:])

    prev = None
    for j in range(NQ):
        queue = list(j0_fill) if j == 0 else list(jn_fill[j])
        n_early = len(queue)
        if prev is not None:
            queue += outproj_units(prev[0], prev[1])
        pops_per_chunk = 4 if j == 0 else 1
        skip_chunks = 0 if j == 0 else 2  # let the norm chain drain first
        qi = 0
        ctxn = [ctxp.tile([P, 512], bf16, tag=f"ctxn{f}", name=f"ctxn{f}")
                for f in range(FK)]
        ctx_ps_h = {}
        exb = {}           # (h, c) -> ex tile (deferred-consumption buffers)
        pending = []       # ctx chunk work: (h, c)
        done_ctx = {h: 0 for h in range(NHC)}

        def emit_ctx(h, c):
            if h not in ctx_ps_h:
                ctx_ps_h[h] = acc_ps.tile([VW, 512], f32, tag="acc",
                                          name=f"ctxps{h}")
            cp = ctx_ps_h[h]
            ex = exb.pop((h, c))
            for t in range(2):
                kt2 = 2 * c + t
                nc.tensor.matmul(
                    cp[:], vsb[kt2][:, h * VW:(h + 1) * VW],
                    ex[:, t * 512:(t + 1) * 512],
                    start=(kt2 == 0), stop=(kt2 == SK - 1))
            done_ctx[h] += 1
            if done_ctx[h] == NCH:
                norm_head(h, cp, ctxn)

        def vsb_ready_upto(popped):
            if j != 0:
                return 10 ** 9
            vp = max(0, popped - n_vp_before)
            return 2 * (vp // GPP) - 1  # v pairs completed -> last tile idx

        for h in range(NHC):
            qv = qT[h // 2][:, j * 512:(j + 1) * 512]
            for c2 in range(NCH // 2):
                if j == 0:
                    # chunk 2*c2+1 reads kz cols of ncol c2: its writer is
                    # queue index 9*c2-1.  Force-pop to keep emission order
                    # = dependency order (Tile deps are trace-order).
                    while qi < min(9 * c2, len(queue)):
                        queue[qi]()
                        qi += 1
                # score burst: 4 matmuls sharing the same moving qv
                scs = []
                for c in (2 * c2, 2 * c2 + 1):
                    sc = sc_ps.tile([P, 2 * 512], f32, tag="sc")
                    for t in range(2):
                        kt2 = 2 * c + t
                        nc.tensor.matmul(
                            sc[:, t * 512:(t + 1) * 512],
                            kz[h][:, kt2 * P:(kt2 + 1) * P],
                            qv, start=True, stop=True)
                    scs.append(sc)
                for i, c in enumerate((2 * c2, 2 * c2 + 1)):
                    ex = exp.tile([P, 2 * 512], bf16, tag="ex")
                    nc.scalar.activation(ex[:], scs[i][:], EXP,
                                         scale=1.0 / (HD ** 0.5))
                    exb[(h, c)] = ex
                    pending.append((h, c))
                    gchunk = h * NCH + c
                    if gchunk >= skip_chunks or qi < n_early:
                        for _ in range(pops_per_chunk):
                            if qi < len(queue):
                                queue[qi]()
                                qi += 1
                    tmax = vsb_ready_upto(qi)
                    while pending and 2 * pending[0][1] + 1 <= tmax:
                        emit_ctx(*pending.pop(0))
        while qi < len(queue):
            queue[qi]()
            qi += 1
        while pending:
            emit_ctx(*pending.pop(0))
        prev = (j, ctxn)
    for u in outproj_units(prev[0], prev[1], last=True):
        u()


_CACHE = {}


def _build():
    if "nc" in _CACHE:
        return _CACHE["nc"]
    nc = bacc.Bacc("TRN2", target_bir_lowering=False, debug=False)
    QT = nc.dram_tensor("QT", [D, S], f8, kind="ExternalInput").ap()
    KT = nc.dram_tensor("KT", [D, S], f8, kind="ExternalInput").ap()
    VT = nc.dram_tensor("VT", [D, S], bf16, kind="ExternalInput").ap()
    WqT = nc.dram_tensor("WqT", [D, FS], f8, kind="ExternalInput").ap()
    WkT = nc.dram_tensor("WkT", [D, FS], f8, kind="ExternalInput").ap()
    WvT = nc.dram_tensor("WvT", [D, FS], bf16, kind="ExternalInput").ap()
    WoT = nc.dram_tensor("WoT", [FS, D], bf16, kind="ExternalInput").ap()
    bq = nc.dram_tensor("bq", [FS, 1], f32, kind="ExternalInput").ap()
    bk = nc.dram_tensor("bk", [FS, 1], f32, kind="ExternalInput").ap()
    bv = nc.dram_tensor("bv", [1, FS], f32, kind="ExternalInput").ap()
    OUTP = nc.dram_tensor("OUTP", [S, D], f16, kind="ExternalOutput").ap()
    with tile.TileContext(nc) as tc, ExitStack() as ctx:
        _emit(ctx, tc, nc, (QT, KT, VT, WqT, WkT, WvT, WoT, bq, bk, bv, OUTP))
    nc.compile()
    _CACHE["nc"] = nc
    return nc


def _in_maps(Q, K, V, Wq, bq, Wk, bk, Wv, bv, Wo, bo):
    bf = ml_dtypes.bfloat16
    e4 = ml_dtypes.float8_e4m3

    def cT(a, dt=bf):  # contiguous transpose + cast
        return np.ascontiguousarray(np.asarray(a).T).astype(dt)

    QTb = [cT(Q[b], e4) for b in range(B)]
    KTb = [cT(K[b], e4) for b in range(B)]
    VTb = [cT(V[b]) for b in range(B)]
    c = np.ascontiguousarray
    maps = []
    for core in range(8):
        b, g = divmod(core, NG)
        sl = slice(g * FS, (g + 1) * FS)
        maps.append({
            "QT": QTb[b], "KT": KTb[b], "VT": VTb[b],
            "WqT": cT(np.asarray(Wq)[sl, :] * WS, e4),
            "WkT": cT(np.asarray(Wk)[sl, :] * WS, e4),
            "WvT": cT(Wv[sl, :]), "WoT": cT(Wo[:, sl]),
            "bq": c(np.asarray(bq)[sl].reshape(FS, 1)),
            "bk": c(np.asarray(bk)[sl].reshape(FS, 1)),
            "bv": c(np.asarray(bv)[sl].reshape(1, FS)),
        })
    return maps


def kernel(Q, K, V, Wq, bq, Wk, bk, Wv, bv, Wo, bo):
    nc = _build()
    maps = _in_maps(Q, K, V, Wq, bq, Wk, bk, Wv, bv, Wo, bo)
    res = run_bass_kernel_spmd(nc, maps, core_ids=list(range(8)))
    out = np.empty((B, S, D), np.float32)
    for b in range(B):
        acc = res.results[b * NG]["OUTP"].astype(np.float32)
        for g in range(1, NG):
            acc = acc + res.results[b * NG + g]["OUTP"].astype(np.float32)
        out[b] = acc + np.asarray(bo, np.float32)[None, :]
    return out
